# revision 28
# baseline (speedup 1.0000x reference)
"""GQA causal attention on 8 TRN2 NeuronCores.

Reference (B=2, T=2048, D=2048, 32 q-heads, 8 kv-heads, dh=64):
    q = x@wq.T, k = x@wk.T, v = x@wv.T  (GQA expand k/v 4x)
    out = softmax(q@k.T/8 + causal) @ v, concat heads, @ wo.T

Sharding: tensor-parallel over heads; core c owns q-heads [4c,4c+4) and
kv-head c. No on-device collectives: the output projection is row-parallel
(each core multiplies its own 256 context features into a full-size
partial), and the host sums the 8 partials (the "all-reduce" of the
hint, performed at unshard time).

Per-core pipeline (PE compute in fp16; accumulation fp32 in PSUM):
  1. x^T streamed in d-major so every contraction is partition-axis.
     Q^T proj packs 2 heads per matmul (M=128); K^T/V^T share one matmul.
  2. K^T duplicated to partitions 64:128 so the two heads of a pair run
     their score matmuls concurrently via PE row-tiling (K=64).
  3. S^T[k,q] tiles from matmul -> exp on ACT (scores are O(3), no max
     subtraction needed) -> causal masking of the 4 diagonal tiles by
     gpsimd affine_select (computed, no mask tensors).
  4. PV matmul with V augmented by a ones column: row 64 of the PSUM
     accumulator is the softmax denominator for free.
  5. Denominators for all 4 (head, half) combos of a q-block are copied
     to PSUM-aligned partitions {0,32,64,96} of one tile -> ONE batched
     reciprocal -> partition_broadcast -> fused normalize into ctx^T fp16.
  6. Row-parallel out^T partial = wo_c^T-slices @ ctx^T, written as
     [B, 2048, T]; host sums cores and transposes.
"""

import sys

sys.path.insert(0, "/opt/trn_rl_repo")

import numpy as np

import concourse.bass as bass  # noqa: F401
import concourse.mybir as mybir
import concourse.tile as tile
from concourse.masks import make_identity
from concourse import bacc
from concourse.bass_utils import run_bass_kernel_spmd

F32 = mybir.dt.float32
F16 = mybir.dt.float16
EXP = mybir.ActivationFunctionType.Exp

B, T, D = 2, 2048, 2048
H, KVH, DH = 32, 8, 64
NC = 8
HPC = H // NC  # 4 q-heads per core
PAIRS = HPC // 2
QB = 512
KB = 128
NJ = T // QB
NKK = T // KB
ND = D // 128
VW = DH + 1

_CACHE = {}


def _build():
    nc = bacc.Bacc("TRN2", target_bir_lowering=False, debug=False, num_devices=NC)

    xT = nc.dram_tensor("xT", [B, D, T], F16, kind="ExternalInput").ap()
    wqT = nc.dram_tensor("wqT", [D, 256], F16, kind="ExternalInput").ap()
    wkvT = nc.dram_tensor("wkvT", [D, 128], F16, kind="ExternalInput").ap()
    woT = nc.dram_tensor("woT", [256, D], F16, kind="ExternalInput").ap()
    masks = nc.dram_tensor("masks", [4, 128, QB], F16, kind="ExternalInput").ap()
    outT = nc.dram_tensor("outT", [B, D, T], F32, kind="ExternalOutput").ap()

    with tile.TileContext(nc) as tc:
        with (
            tc.tile_pool(name="const", bufs=1) as constp,
            tc.tile_pool(name="xstrip", bufs=4) as xtp,
            tc.tile_pool(name="ktp", bufs=2) as ktp,
            tc.tile_pool(name="vvp", bufs=2) as vvp,
            tc.tile_pool(name="qtp", bufs=3) as qtp,
            tc.tile_pool(name="esp", bufs=5) as esp,
            tc.tile_pool(name="small", bufs=3) as smallp,
            tc.tile_pool(name="oup", bufs=4) as oup,
            tc.tile_pool(name="rrp", bufs=2) as rrp,
            tc.tile_pool(name="ctxlp", bufs=2) as ctxlp,
            tc.tile_pool(name="ofp", bufs=3) as ofp,
            tc.tile_pool(name="ps_s", bufs=2, space="PSUM") as psp,
            tc.tile_pool(name="ps_proj", bufs=2, space="PSUM") as prjp,
            tc.tile_pool(name="ps_o", bufs=1, space="PSUM") as pop,
        ):
            # ---- constants ----
            wq_sb = constp.tile([128, ND, 256], F16)
            wqs = wqT.rearrange("(n p) m -> p n m", p=128)
            nc.scalar.dma_start(out=wq_sb[:, 0:4, :], in_=wqs[:, 0:4, :])
            nc.scalar.dma_start(out=wq_sb[:, 4:16, :], in_=wqs[:, 4:16, :])
            wkv_sb = constp.tile([128, ND, 128], F16)
            wkvs = wkvT.rearrange("(n p) m -> p n m", p=128)
            nc.gpsimd.dma_start(out=wkv_sb[:, 0:4, :], in_=wkvs[:, 0:4, :])
            nc.gpsimd.dma_start(out=wkv_sb[:, 4:16, :], in_=wkvs[:, 4:16, :])
            wo_sb = constp.tile([128, 2, D], F16)
            wo_loaded = [False]

            def load_wo():
                if not wo_loaded[0]:
                    nc.scalar.dma_start(
                        out=wo_sb[:], in_=woT.rearrange("(g p) m -> p g m", p=128)
                    )
                    wo_loaded[0] = True
            ident = constp.tile([128, 128], F32)
            make_identity(nc, ident[:])
            mask_sb = constp.tile([128, 4, QB], F16)
            nc.scalar.dma_start(out=mask_sb[:], in_=masks.rearrange("m p q -> p m q"))
            onecol = constp.tile([97, 64], F16)
            nc.vector.memset(onecol[:], 1.0)

            def emit_fp(b, tt):
                # row-parallel output projection for one query block
                load_wo()
                ctxl = ctxls[b]
                for och in range(4):
                    of = ofp.tile([128, 4, QB], F32, tag="of")
                    for oc2 in range(2):
                        pf = psp.tile([128, 2 * QB], F32, tag="ps", name="pf")
                        for sub in range(2):
                            oc0 = (och * 4 + oc2 * 2 + sub) * 128
                            for g in range(2):
                                nc.tensor.matmul(
                                    pf[:, sub * QB : (sub + 1) * QB],
                                    wo_sb[:, g, oc0 : oc0 + 128],
                                    ctxl[:, g, tt * QB : (tt + 1) * QB],
                                    start=(g == 0),
                                    stop=(g == 1),
                                )
                        ofv = of[:, oc2 * 2 : (oc2 + 1) * 2, :]
                        if oc2 % 2 == 0:
                            nc.vector.tensor_copy(ofv, pf[:])
                        else:
                            nc.scalar.copy(ofv, pf[:])
                    nc.gpsimd.dma_start(
                        out=outT[
                            b, och * 512 : (och + 1) * 512, tt * QB : (tt + 1) * QB
                        ].rearrange("(n p) q -> p n q", p=128),
                        in_=of[:],
                    )

            fp_queue = []  # (b, tt) ready to emit, one block behind attention
            # keep PE busy+warm while the first DMAs land
            junk = constp.tile([128, QB], F16, name="junk")
            nc.vector.memset(junk[:], 0.001)
            for w in range(50):
                pw = pop.tile([64, QB], F32, tag="po", name="pw")
                nc.tensor.matmul(
                    pw[:], junk[0:64, 0:64], junk[0:64, :], start=True, stop=True
                )

            ctxls = []
            for b in range(B):
                kt = ktp.tile([128, T], F16, tag="kt")  # K^T on both halves
                vv = vvp.tile([128, NKK * VW], F16, tag="vv")  # [V | 1] tiles
                nc.vector.memset(
                    vv[:].rearrange("p (n c) -> p n c", c=VW)[:, :, DH : DH + 1], 1.0
                )
                ctxl = ctxlp.tile([128, 2, T], F16, tag="ctxl")  # normalized ctx^T
                ctxls.append(ctxl)

                xts = []
                for j in range(NJ):
                    xt = xtp.tile([128, ND, QB], F16, tag="xstrip", name=f"xt{b}{j}")
                    xsrc = xT[b, :, j * QB : (j + 1) * QB].rearrange(
                        "(n p) q -> p n q", p=128
                    )
                    bounds = [0, 1, 2, 4, 8, 16] if (b == 0 and j == 0) else [0, 4, 8, 12, 16]
                    for lo, hi in zip(bounds[:-1], bounds[1:]):
                        nc.sync.dma_start(
                            out=xt[:, lo:hi, :], in_=xsrc[:, lo:hi, :]
                        )
                    xts.append(xt)

                for j in range(NJ):
                    xt = xts[j]

                    # Q^T projection, two heads per matmul (M=128)
                    qts = []
                    for p in range(PAIRS):
                        pq = prjp.tile([128, QB], F32, tag="proj")
                        for dk in range(ND):
                            nc.tensor.matmul(
                                pq[:],
                                wq_sb[:, dk, p * 128 : (p + 1) * 128],
                                xt[:, dk, :],
                                start=(dk == 0),
                                stop=(dk == ND - 1),
                            )
                        qt = qtp.tile([128, QB], F16, tag="qt")
                        nc.vector.tensor_copy(qt[:], pq[:])
                        qts.append(qt)

                    # K^T rows 0:64, V^T rows 64:128 in one accumulation
                    pkv = prjp.tile([128, QB], F32, tag="proj")
                    for dk in range(ND):
                        nc.tensor.matmul(
                            pkv[:],
                            wkv_sb[:, dk, :],
                            xt[:, dk, :],
                            start=(dk == 0),
                            stop=(dk == ND - 1),
                        )
                    nc.vector.tensor_copy(kt[0:64, j * QB : (j + 1) * QB], pkv[0:64, :])
                    nc.vector.tensor_copy(
                        kt[64:128, j * QB : (j + 1) * QB], pkv[0:64, :]
                    )
                    vt_sb = smallp.tile([64, QB], F32, tag="vt")
                    nc.scalar.copy(vt_sb[:], pkv[64:128, :])
                    for i in range(4):
                        pvt = prjp.tile([128, 64], F32, tag="proj")
                        nc.tensor.transpose(
                            pvt[:], vt_sb[:, i * 128 : (i + 1) * 128], ident[0:64, 0:64]
                        )
                        kk = 4 * j + i
                        nc.vector.tensor_copy(vv[:, kk * VW : kk * VW + DH], pvt[:])

                    if fp_queue and len(fp_queue) > 1:
                        emit_fp(*fp_queue.pop(0))

                    # attention for this query block
                    rr4 = rrp.tile([97, QB], F32, tag="rr4")
                    ous = []
                    for p in range(PAIRS):
                        po = pop.tile([65, 2 * QB], F32, tag="po")
                        nkk = 4 * (j + 1)

                        def emit_scores(kk, p=p):
                            m = kk - 4 * j
                            q0 = max(0, m) * KB  # masked-out query prefix
                            qn = QB - q0
                            ps = psp.tile([128, 2 * QB], F32, tag="ps", name="ps")
                            nc.tensor.matmul(
                                ps[:, q0:QB],
                                kt[0:64, kk * KB : (kk + 1) * KB],
                                qts[p][0:64, q0:QB],
                                start=True,
                                stop=True,
                            )
                            nc.tensor.matmul(
                                ps[:, QB + q0 : 2 * QB],
                                kt[64:128, kk * KB : (kk + 1) * KB],
                                qts[p][64:128, q0:QB],
                                start=True,
                                stop=True,
                                tile_position=(64, 0),
                            )
                            es = esp.tile([128, 2 * QB], F16, tag="es", name="es")
                            if q0 <= KB:
                                nc.scalar.activation(es[:], ps[:], EXP)
                            else:
                                nc.scalar.activation(es[:, q0:QB], ps[:, q0:QB], EXP)
                                nc.scalar.activation(
                                    es[:, QB + q0 : 2 * QB], ps[:, QB + q0 : 2 * QB], EXP
                                )
                            if m >= 0:
                                esv = es[:].rearrange("p (h q) -> p h q", q=QB)[
                                    :, :, q0:QB
                                ]
                                nc.vector.tensor_mul(
                                    esv,
                                    esv,
                                    mask_sb[:, m : m + 1, q0:QB].broadcast_to(
                                        [128, 2, qn]
                                    ),
                                )
                            return es, q0

                        pipe = [emit_scores(0)]
                        if nkk > 1:
                            pipe.append(emit_scores(1))
                        for kk in range(nkk):
                            es_cur, q0 = pipe.pop(0)
                            if kk + 2 < nkk:
                                pipe.append(emit_scores(kk + 2))
                            for hh in range(2):
                                nc.tensor.matmul(
                                    po[0:65, hh * QB + q0 : (hh + 1) * QB],
                                    vv[:, kk * VW : (kk + 1) * VW],
                                    es_cur[:, hh * QB + q0 : (hh + 1) * QB],
                                    start=(kk == 0),
                                    stop=(kk == nkk - 1),
                                )
                        # free PSUM fast: row sums to rr4, O' to SBUF
                        nc.vector.tensor_copy(rr4[64 * p : 64 * p + 1, :], po[64:65, 0:QB])
                        nc.vector.tensor_copy(
                            rr4[64 * p + 32 : 64 * p + 33, :], po[64:65, QB : 2 * QB]
                        )
                        ou = oup.tile([64, 2 * QB], F32, tag="ou")
                        nc.vector.tensor_copy(ou[:], po[0:64, :])
                        ous.append(ou)
                    # one reciprocal for all 4 (head, half) combos of block j
                    ir4f = rrp.tile([97, QB], F32, tag="ir4f")
                    nc.vector.reciprocal_approx_fast(ir4f[:], rr4[:])
                    ir4 = rrp.tile([97, QB], F16, tag="ir4")
                    nc.vector.tensor_copy(ir4[:], ir4f[:])
                    for p in range(PAIRS):
                        for hh in range(2):
                            k32 = 64 * p + 32 * hh
                            pb = pop.tile([64, QB], F32, tag="po")
                            nc.tensor.matmul(
                                pb[:],
                                onecol[k32 : k32 + 1, :],
                                ir4[k32 : k32 + 1, :],
                                start=True,
                                stop=True,
                                tile_position=(k32, 0),
                            )
                            hidx = 2 * p + hh
                            nc.vector.tensor_mul(
                                ctxl[
                                    64 * (hidx % 2) : 64 * (hidx % 2) + 64,
                                    hidx // 2,
                                    j * QB : (j + 1) * QB,
                                ],
                                ous[p][:, hh * QB : (hh + 1) * QB],
                                pb[:],
                            )

                    fp_queue.append((b, j))
            while fp_queue:
                emit_fp(*fp_queue.pop(0))
    nc.finalize()
    return nc


def _prep_in_maps(x, wq, wk, wv, wo):
    xT = np.ascontiguousarray(x.transpose(0, 2, 1)).astype(np.float16)
    k_idx = np.arange(128)[:, None]
    q_idx = np.arange(QB)[None, :]
    masks = np.stack(
        [(128 * m + k_idx <= q_idx).astype(np.float16) for m in range(4)]
    )
    in_maps = []
    for c in range(NC):
        wq_c = (wq[c * 256 : (c + 1) * 256] * np.float32(DH ** -0.5)).astype(np.float16)
        wkv_c = np.concatenate(
            [wk[c * DH : (c + 1) * DH], wv[c * DH : (c + 1) * DH]], axis=0
        ).astype(np.float16)
        wo_c = wo[:, c * 256 : (c + 1) * 256].astype(np.float16)  # [2048, 256]
        in_maps.append(
            {
                "xT": xT,
                "wqT": np.ascontiguousarray(wq_c.T),
                "wkvT": np.ascontiguousarray(wkv_c.T),
                "woT": np.ascontiguousarray(wo_c.T),  # [256, 2048]
                "masks": masks,
            }
        )
    return in_maps


def run(inputs, trace=False, trace_kwargs=None):
    if "nc" not in _CACHE:
        _CACHE["nc"] = _build()
    nc = _CACHE["nc"]
    in_maps = _prep_in_maps(
        np.asarray(inputs["x"], np.float32),
        np.asarray(inputs["wq"], np.float32),
        np.asarray(inputs["wk"], np.float32),
        np.asarray(inputs["wv"], np.float32),
        np.asarray(inputs["wo"], np.float32),
    )
    res = run_bass_kernel_spmd(
        nc,
        in_maps,
        core_ids=list(range(NC)),
        trace=trace,
        **(trace_kwargs or {}),
    )
    acc = np.zeros((B, D, T), np.float32)
    for r in res.results:
        acc += r["outT"]
    full = acc.transpose(0, 2, 1)
    return np.ascontiguousarray(full), res


def kernel(**inputs) -> np.ndarray:
    out, _ = run(inputs, trace=False)
    return out


# revision 29
# speedup vs baseline: 1.0048x; 1.0048x over previous
"""GQA causal attention on 8 TRN2 NeuronCores.

Reference (B=2, T=2048, D=2048, 32 q-heads, 8 kv-heads, dh=64):
    q = x@wq.T, k = x@wk.T, v = x@wv.T  (GQA expand k/v 4x)
    out = softmax(q@k.T/8 + causal) @ v, concat heads, @ wo.T

Sharding: tensor-parallel over heads; core c owns q-heads [4c,4c+4) and
kv-head c. No on-device collectives: the output projection is row-parallel
(each core multiplies its own 256 context features into a full-size
partial), and the host sums the 8 partials (the "all-reduce" of the
hint, performed at unshard time).

Per-core pipeline (PE compute in fp16; accumulation fp32 in PSUM):
  1. x^T streamed in d-major so every contraction is partition-axis.
     Q^T proj packs 2 heads per matmul (M=128); K^T/V^T share one matmul.
  2. K^T duplicated to partitions 64:128 so the two heads of a pair run
     their score matmuls concurrently via PE row-tiling (K=64).
  3. S^T[k,q] tiles from matmul -> exp on ACT (scores are O(3), no max
     subtraction needed) -> causal masking of the 4 diagonal tiles by
     gpsimd affine_select (computed, no mask tensors).
  4. PV matmul with V augmented by a ones column: row 64 of the PSUM
     accumulator is the softmax denominator for free.
  5. Denominators for all 4 (head, half) combos of a q-block are copied
     to PSUM-aligned partitions {0,32,64,96} of one tile -> ONE batched
     reciprocal -> partition_broadcast -> fused normalize into ctx^T fp16.
  6. Row-parallel out^T partial = wo_c^T-slices @ ctx^T, written as
     [B, 2048, T]; host sums cores and transposes.
"""

import sys

sys.path.insert(0, "/opt/trn_rl_repo")

import numpy as np

import concourse.bass as bass  # noqa: F401
import concourse.mybir as mybir
import concourse.tile as tile
from concourse.masks import make_identity
from concourse import bacc
from concourse.bass_utils import run_bass_kernel_spmd

F32 = mybir.dt.float32
F16 = mybir.dt.float16
EXP = mybir.ActivationFunctionType.Exp

B, T, D = 2, 2048, 2048
H, KVH, DH = 32, 8, 64
NC = 8
HPC = H // NC  # 4 q-heads per core
PAIRS = HPC // 2
QB = 512
KB = 128
NJ = T // QB
NKK = T // KB
ND = D // 128
VW = DH + 1

_CACHE = {}


def _build():
    nc = bacc.Bacc("TRN2", target_bir_lowering=False, debug=False, num_devices=NC)

    xT = nc.dram_tensor("xT", [B, D, T], F16, kind="ExternalInput").ap()
    wqT = nc.dram_tensor("wqT", [D, 256], F16, kind="ExternalInput").ap()
    wkvT = nc.dram_tensor("wkvT", [D, 128], F16, kind="ExternalInput").ap()
    woT = nc.dram_tensor("woT", [256, D], F16, kind="ExternalInput").ap()
    masks = nc.dram_tensor("masks", [4, 128, QB], F16, kind="ExternalInput").ap()
    outT = nc.dram_tensor("outT", [B, D, T], F32, kind="ExternalOutput").ap()

    with tile.TileContext(nc) as tc:
        with (
            tc.tile_pool(name="const", bufs=1) as constp,
            tc.tile_pool(name="xstrip", bufs=4) as xtp,
            tc.tile_pool(name="ktp", bufs=2) as ktp,
            tc.tile_pool(name="vvp", bufs=2) as vvp,
            tc.tile_pool(name="qtp", bufs=3) as qtp,
            tc.tile_pool(name="esp", bufs=5) as esp,
            tc.tile_pool(name="small", bufs=3) as smallp,
            tc.tile_pool(name="oup", bufs=4) as oup,
            tc.tile_pool(name="rrp", bufs=2) as rrp,
            tc.tile_pool(name="ctxlp", bufs=2) as ctxlp,
            tc.tile_pool(name="ofp", bufs=3) as ofp,
            tc.tile_pool(name="ps_s", bufs=2, space="PSUM") as psp,
            tc.tile_pool(name="ps_proj", bufs=2, space="PSUM") as prjp,
            tc.tile_pool(name="ps_o", bufs=1, space="PSUM") as pop,
        ):
            # ---- constants ----
            wq_sb = constp.tile([128, ND, 256], F16)
            wqs = wqT.rearrange("(n p) m -> p n m", p=128)
            nc.scalar.dma_start(out=wq_sb[:, 0:4, :], in_=wqs[:, 0:4, :])
            nc.scalar.dma_start(out=wq_sb[:, 4:16, :], in_=wqs[:, 4:16, :])
            wkv_sb = constp.tile([128, ND, 128], F16)
            wkvs = wkvT.rearrange("(n p) m -> p n m", p=128)
            nc.gpsimd.dma_start(out=wkv_sb[:, 0:4, :], in_=wkvs[:, 0:4, :])
            nc.gpsimd.dma_start(out=wkv_sb[:, 4:16, :], in_=wkvs[:, 4:16, :])
            wo_sb = constp.tile([128, 2, D], F16)
            wo_loaded = [False]

            def load_wo():
                if not wo_loaded[0]:
                    nc.scalar.dma_start(
                        out=wo_sb[:], in_=woT.rearrange("(g p) m -> p g m", p=128)
                    )
                    wo_loaded[0] = True
            ident = constp.tile([128, 128], F32)
            make_identity(nc, ident[:])
            mask_sb = constp.tile([128, 4, QB], F16)
            nc.scalar.dma_start(out=mask_sb[:], in_=masks.rearrange("m p q -> p m q"))
            onecol = constp.tile([97, 64], F16)
            nc.vector.memset(onecol[:], 1.0)

            def emit_fp(b, tt):
                # row-parallel output projection for one query block
                load_wo()
                ctxl = ctxls[b]
                for och in range(4):
                    of = ofp.tile([128, 4, QB], F32, tag="of")
                    for oc2 in range(2):
                        pf = psp.tile([128, 2 * QB], F32, tag="ps", name="pf")
                        for sub in range(2):
                            oc0 = (och * 4 + oc2 * 2 + sub) * 128
                            for g in range(2):
                                nc.tensor.matmul(
                                    pf[:, sub * QB : (sub + 1) * QB],
                                    wo_sb[:, g, oc0 : oc0 + 128],
                                    ctxl[:, g, tt * QB : (tt + 1) * QB],
                                    start=(g == 0),
                                    stop=(g == 1),
                                )
                        ofv = of[:, oc2 * 2 : (oc2 + 1) * 2, :]
                        if oc2 % 2 == 0:
                            nc.vector.tensor_copy(ofv, pf[:])
                        else:
                            nc.scalar.copy(ofv, pf[:])
                    nc.gpsimd.dma_start(
                        out=outT[
                            b, och * 512 : (och + 1) * 512, tt * QB : (tt + 1) * QB
                        ].rearrange("(n p) q -> p n q", p=128),
                        in_=of[:],
                    )

            fp_queue = []  # (b, tt) ready to emit, one block behind attention
            # keep PE busy+warm while the first DMAs land
            junk = constp.tile([128, QB], F16, name="junk")
            nc.vector.memset(junk[:], 0.001)
            for w in range(50):
                pw = pop.tile([64, QB], F32, tag="po", name="pw")
                nc.tensor.matmul(
                    pw[:], junk[0:64, 0:64], junk[0:64, :], start=True, stop=True
                )

            ctxls = []
            for b in range(B):
                kt = ktp.tile([128, T], F16, tag="kt")  # K^T on both halves
                vv = vvp.tile([128, NKK * VW], F16, tag="vv")  # [V | 1] tiles
                nc.vector.memset(
                    vv[:].rearrange("p (n c) -> p n c", c=VW)[:, :, DH : DH + 1], 1.0
                )
                ctxl = ctxlp.tile([128, 2, T], F16, tag="ctxl")  # normalized ctx^T
                ctxls.append(ctxl)

                xts = []
                for j in range(NJ):
                    xt = xtp.tile([128, ND, QB], F16, tag="xstrip", name=f"xt{b}{j}")
                    xsrc = xT[b, :, j * QB : (j + 1) * QB].rearrange(
                        "(n p) q -> p n q", p=128
                    )
                    bounds = [0, 1, 2, 4, 8, 16] if (b == 0 and j == 0) else [0, 4, 8, 12, 16]
                    for lo, hi in zip(bounds[:-1], bounds[1:]):
                        nc.sync.dma_start(
                            out=xt[:, lo:hi, :], in_=xsrc[:, lo:hi, :]
                        )
                    xts.append(xt)

                for j in range(NJ):
                    xt = xts[j]

                    # Q^T projection, two heads per matmul (M=128)
                    qts = []
                    for p in range(PAIRS):
                        pq = prjp.tile([128, QB], F32, tag="proj")
                        for dk in range(ND):
                            nc.tensor.matmul(
                                pq[:],
                                wq_sb[:, dk, p * 128 : (p + 1) * 128],
                                xt[:, dk, :],
                                start=(dk == 0),
                                stop=(dk == ND - 1),
                            )
                        qt = qtp.tile([128, QB], F16, tag="qt")
                        nc.vector.tensor_copy(qt[:], pq[:])
                        qts.append(qt)

                    # K^T rows 0:64, V^T rows 64:128 in one accumulation
                    pkv = prjp.tile([128, QB], F32, tag="proj")
                    for dk in range(ND):
                        nc.tensor.matmul(
                            pkv[:],
                            wkv_sb[:, dk, :],
                            xt[:, dk, :],
                            start=(dk == 0),
                            stop=(dk == ND - 1),
                        )
                    nc.vector.tensor_copy(kt[0:64, j * QB : (j + 1) * QB], pkv[0:64, :])
                    nc.vector.tensor_copy(
                        kt[64:128, j * QB : (j + 1) * QB], pkv[0:64, :]
                    )
                    vt_sb = smallp.tile([64, QB], F32, tag="vt")
                    nc.scalar.copy(vt_sb[:], pkv[64:128, :])
                    for i in range(4):
                        pvt = prjp.tile([128, 64], F32, tag="proj")
                        nc.tensor.transpose(
                            pvt[:], vt_sb[:, i * 128 : (i + 1) * 128], ident[0:64, 0:64]
                        )
                        kk = 4 * j + i
                        nc.vector.tensor_copy(vv[:, kk * VW : kk * VW + DH], pvt[:])

                    if fp_queue and len(fp_queue) > 1:
                        emit_fp(*fp_queue.pop(0))

                    # attention for this query block
                    rr4 = rrp.tile([97, QB], F32, tag="rr4")
                    ous = []
                    for p in range(PAIRS):
                        po = pop.tile([65, 2 * QB], F32, tag="po")
                        nkk = 4 * (j + 1)

                        def emit_scores(kk, p=p):
                            m = kk - 4 * j
                            q0 = max(0, m) * KB  # masked-out query prefix
                            qn = QB - q0
                            ps = psp.tile([128, 2 * QB], F32, tag="ps", name="ps")
                            nc.tensor.matmul(
                                ps[:, q0:QB],
                                kt[0:64, kk * KB : (kk + 1) * KB],
                                qts[p][0:64, q0:QB],
                                start=True,
                                stop=True,
                            )
                            nc.tensor.matmul(
                                ps[:, QB + q0 : 2 * QB],
                                kt[64:128, kk * KB : (kk + 1) * KB],
                                qts[p][64:128, q0:QB],
                                start=True,
                                stop=True,
                                tile_position=(64, 0),
                            )
                            es = esp.tile([128, 2 * QB], F16, tag="es", name="es")
                            if q0 <= KB:
                                nc.scalar.activation(es[:], ps[:], EXP)
                            else:
                                nc.scalar.activation(es[:, q0:QB], ps[:, q0:QB], EXP)
                                nc.scalar.activation(
                                    es[:, QB + q0 : 2 * QB], ps[:, QB + q0 : 2 * QB], EXP
                                )
                            if m >= 0:
                                esv = es[:].rearrange("p (h q) -> p h q", q=QB)[
                                    :, :, q0:QB
                                ]
                                nc.vector.tensor_mul(
                                    esv,
                                    esv,
                                    mask_sb[:, m : m + 1, q0:QB].broadcast_to(
                                        [128, 2, qn]
                                    ),
                                )
                            return es, q0

                        es_next = emit_scores(0)
                        for kk in range(nkk):
                            es_cur, q0 = es_next
                            if kk + 1 < nkk:
                                es_next = emit_scores(kk + 1)
                            for hh in range(2):
                                nc.tensor.matmul(
                                    po[0:65, hh * QB + q0 : (hh + 1) * QB],
                                    vv[:, kk * VW : (kk + 1) * VW],
                                    es_cur[:, hh * QB + q0 : (hh + 1) * QB],
                                    start=(kk == 0),
                                    stop=(kk == nkk - 1),
                                )
                        # free PSUM fast: row sums to rr4, O' to SBUF
                        nc.vector.tensor_copy(rr4[64 * p : 64 * p + 1, :], po[64:65, 0:QB])
                        nc.vector.tensor_copy(
                            rr4[64 * p + 32 : 64 * p + 33, :], po[64:65, QB : 2 * QB]
                        )
                        ou = oup.tile([64, 2 * QB], F32, tag="ou")
                        nc.vector.tensor_copy(ou[:], po[0:64, :])
                        ous.append(ou)
                    # one reciprocal for all 4 (head, half) combos of block j
                    ir4f = rrp.tile([97, QB], F32, tag="ir4f")
                    nc.vector.reciprocal_approx_fast(ir4f[:], rr4[:])
                    ir4 = rrp.tile([97, QB], F16, tag="ir4")
                    nc.vector.tensor_copy(ir4[:], ir4f[:])
                    for p in range(PAIRS):
                        for hh in range(2):
                            k32 = 64 * p + 32 * hh
                            pb = pop.tile([64, QB], F32, tag="po")
                            nc.tensor.matmul(
                                pb[:],
                                onecol[k32 : k32 + 1, :],
                                ir4[k32 : k32 + 1, :],
                                start=True,
                                stop=True,
                                tile_position=(k32, 0),
                            )
                            hidx = 2 * p + hh
                            nc.vector.tensor_mul(
                                ctxl[
                                    64 * (hidx % 2) : 64 * (hidx % 2) + 64,
                                    hidx // 2,
                                    j * QB : (j + 1) * QB,
                                ],
                                ous[p][:, hh * QB : (hh + 1) * QB],
                                pb[:],
                            )

                    fp_queue.append((b, j))
            while fp_queue:
                emit_fp(*fp_queue.pop(0))
    nc.finalize()
    return nc


def _prep_in_maps(x, wq, wk, wv, wo):
    xT = np.ascontiguousarray(x.transpose(0, 2, 1)).astype(np.float16)
    k_idx = np.arange(128)[:, None]
    q_idx = np.arange(QB)[None, :]
    masks = np.stack(
        [(128 * m + k_idx <= q_idx).astype(np.float16) for m in range(4)]
    )
    in_maps = []
    for c in range(NC):
        wq_c = (wq[c * 256 : (c + 1) * 256] * np.float32(DH ** -0.5)).astype(np.float16)
        wkv_c = np.concatenate(
            [wk[c * DH : (c + 1) * DH], wv[c * DH : (c + 1) * DH]], axis=0
        ).astype(np.float16)
        wo_c = wo[:, c * 256 : (c + 1) * 256].astype(np.float16)  # [2048, 256]
        in_maps.append(
            {
                "xT": xT,
                "wqT": np.ascontiguousarray(wq_c.T),
                "wkvT": np.ascontiguousarray(wkv_c.T),
                "woT": np.ascontiguousarray(wo_c.T),  # [256, 2048]
                "masks": masks,
            }
        )
    return in_maps


def run(inputs, trace=False, trace_kwargs=None):
    if "nc" not in _CACHE:
        _CACHE["nc"] = _build()
    nc = _CACHE["nc"]
    in_maps = _prep_in_maps(
        np.asarray(inputs["x"], np.float32),
        np.asarray(inputs["wq"], np.float32),
        np.asarray(inputs["wk"], np.float32),
        np.asarray(inputs["wv"], np.float32),
        np.asarray(inputs["wo"], np.float32),
    )
    res = run_bass_kernel_spmd(
        nc,
        in_maps,
        core_ids=list(range(NC)),
        trace=trace,
        **(trace_kwargs or {}),
    )
    acc = np.zeros((B, D, T), np.float32)
    for r in res.results:
        acc += r["outT"]
    full = acc.transpose(0, 2, 1)
    return np.ascontiguousarray(full), res


def kernel(**inputs) -> np.ndarray:
    out, _ = run(inputs, trace=False)
    return out


# revision 30
# speedup vs baseline: 1.0919x; 1.0868x over previous
"""GQA causal attention on 8 TRN2 NeuronCores.

Reference (B=2, T=2048, D=2048, 32 q-heads, 8 kv-heads, dh=64):
    q = x@wq.T, k = x@wk.T, v = x@wv.T  (GQA expand k/v 4x)
    out = softmax(q@k.T/8 + causal) @ v, concat heads, @ wo.T

Sharding: tensor-parallel over heads; core c owns q-heads [4c,4c+4) and
kv-head c. No on-device collectives: the output projection is row-parallel
(each core multiplies its own 256 context features into a full-size
partial), and the host sums the 8 partials (the "all-reduce" of the
hint, performed at unshard time).

Per-core pipeline (PE compute in fp16; accumulation fp32 in PSUM):
  1. x^T streamed in d-major so every contraction is partition-axis.
     Q^T proj packs 2 heads per matmul (M=128); K^T/V^T share one matmul.
  2. K^T duplicated to partitions 64:128 so the two heads of a pair run
     their score matmuls concurrently via PE row-tiling (K=64).
  3. S^T[k,q] tiles from matmul -> exp on ACT (scores are O(3), no max
     subtraction needed) -> causal masking of the 4 diagonal tiles by
     gpsimd affine_select (computed, no mask tensors).
  4. PV matmul with V augmented by a ones column: row 64 of the PSUM
     accumulator is the softmax denominator for free.
  5. Denominators for all 4 (head, half) combos of a q-block are copied
     to PSUM-aligned partitions {0,32,64,96} of one tile -> ONE batched
     reciprocal -> partition_broadcast -> fused normalize into ctx^T fp16.
  6. Row-parallel out^T partial = wo_c^T-slices @ ctx^T, written as
     [B, 2048, T]; host sums cores and transposes.
"""

import sys

sys.path.insert(0, "/opt/trn_rl_repo")

import numpy as np

import concourse.bass as bass  # noqa: F401
import concourse.mybir as mybir
import concourse.tile as tile
from concourse.masks import make_identity
from concourse import bacc
from concourse.bass_utils import run_bass_kernel_spmd

F32 = mybir.dt.float32
F16 = mybir.dt.float16
EXP = mybir.ActivationFunctionType.Exp

B, T, D = 2, 2048, 2048
H, KVH, DH = 32, 8, 64
NC = 8
HPC = H // NC  # 4 q-heads per core
PAIRS = HPC // 2
QB = 512
KB = 128
NJ = T // QB
NKK = T // KB
ND = D // 128
VW = DH + 1

_CACHE = {}


def _build():
    nc = bacc.Bacc("TRN2", target_bir_lowering=False, debug=False, num_devices=NC)

    xT = nc.dram_tensor("xT", [B, D, T], F16, kind="ExternalInput").ap()
    wqT = nc.dram_tensor("wqT", [D, 256], F16, kind="ExternalInput").ap()
    wkvT = nc.dram_tensor("wkvT", [D, 128], F16, kind="ExternalInput").ap()
    woT = nc.dram_tensor("woT", [256, D], F16, kind="ExternalInput").ap()
    masks = nc.dram_tensor("masks", [4, 128, QB], F16, kind="ExternalInput").ap()
    outT = nc.dram_tensor("outT", [B, D, T], F32, kind="ExternalOutput").ap()

    with tile.TileContext(nc) as tc:
        with (
            tc.tile_pool(name="const", bufs=1) as constp,
            tc.tile_pool(name="xstrip", bufs=4) as xtp,
            tc.tile_pool(name="ktp", bufs=2) as ktp,
            tc.tile_pool(name="vvp", bufs=2) as vvp,
            tc.tile_pool(name="qtp", bufs=3) as qtp,
            tc.tile_pool(name="esp", bufs=5) as esp,
            tc.tile_pool(name="small", bufs=3) as smallp,
            tc.tile_pool(name="oup", bufs=4) as oup,
            tc.tile_pool(name="rrp", bufs=2) as rrp,
            tc.tile_pool(name="ctxlp", bufs=2) as ctxlp,
            tc.tile_pool(name="ofp", bufs=3) as ofp,
            tc.tile_pool(name="ps_s", bufs=2, space="PSUM") as psp,
            tc.tile_pool(name="ps_proj", bufs=2, space="PSUM") as prjp,
            tc.tile_pool(name="ps_o", bufs=1, space="PSUM") as pop,
        ):
            # ---- constants ----
            wq_sb = constp.tile([128, ND, 256], F16)
            wqs = wqT.rearrange("(n p) m -> p n m", p=128)
            nc.scalar.dma_start(out=wq_sb[:, 0:4, :], in_=wqs[:, 0:4, :])
            nc.scalar.dma_start(out=wq_sb[:, 4:16, :], in_=wqs[:, 4:16, :])
            wkv_sb = constp.tile([128, ND, 128], F16)
            wkvs = wkvT.rearrange("(n p) m -> p n m", p=128)
            nc.gpsimd.dma_start(out=wkv_sb[:, 0:4, :], in_=wkvs[:, 0:4, :])
            nc.gpsimd.dma_start(out=wkv_sb[:, 4:16, :], in_=wkvs[:, 4:16, :])
            wo_sb = constp.tile([128, 2, D], F16)
            wo_loaded = [False]

            def load_wo():
                if not wo_loaded[0]:
                    nc.scalar.dma_start(
                        out=wo_sb[:], in_=woT.rearrange("(g p) m -> p g m", p=128)
                    )
                    wo_loaded[0] = True
            ident = constp.tile([128, 128], F32)
            make_identity(nc, ident[:])
            mask_sb = constp.tile([128, 4, QB], F16)
            nc.scalar.dma_start(out=mask_sb[:], in_=masks.rearrange("m p q -> p m q"))
            onecol = constp.tile([97, 64], F16)
            nc.vector.memset(onecol[:], 1.0)

            def emit_fp(b, tt):
                # row-parallel output projection for one query block
                load_wo()
                ctxl = ctxls[b]
                for och in range(4):
                    of = ofp.tile([128, 4, QB], F32, tag="of")
                    for oc2 in range(2):
                        pf = psp.tile([128, 2 * QB], F32, tag="ps", name="pf")
                        for sub in range(2):
                            oc0 = (och * 4 + oc2 * 2 + sub) * 128
                            for g in range(2):
                                nc.tensor.matmul(
                                    pf[:, sub * QB : (sub + 1) * QB],
                                    wo_sb[:, g, oc0 : oc0 + 128],
                                    ctxl[:, g, tt * QB : (tt + 1) * QB],
                                    start=(g == 0),
                                    stop=(g == 1),
                                )
                        ofv = of[:, oc2 * 2 : (oc2 + 1) * 2, :]
                        if oc2 % 2 == 0:
                            nc.vector.tensor_copy(ofv, pf[:])
                        else:
                            nc.scalar.copy(ofv, pf[:])
                    nc.gpsimd.dma_start(
                        out=outT[
                            b, och * 512 : (och + 1) * 512, tt * QB : (tt + 1) * QB
                        ].rearrange("(n p) q -> p n q", p=128),
                        in_=of[:],
                    )

            fp_queue = []  # (b, tt) ready to emit, one block behind attention
            ctxls = []
            for b in range(B):
                kt = ktp.tile([128, T], F16, tag="kt")  # K^T on both halves
                vv = vvp.tile([128, NKK * VW], F16, tag="vv")  # [V | 1] tiles
                nc.vector.memset(
                    vv[:].rearrange("p (n c) -> p n c", c=VW)[:, :, DH : DH + 1], 1.0
                )
                ctxl = ctxlp.tile([128, 2, T], F16, tag="ctxl")  # normalized ctx^T
                ctxls.append(ctxl)

                xts = []
                for j in range(NJ):
                    xt = xtp.tile([128, ND, QB], F16, tag="xstrip", name=f"xt{b}{j}")
                    xsrc = xT[b, :, j * QB : (j + 1) * QB].rearrange(
                        "(n p) q -> p n q", p=128
                    )
                    bounds = [0, 1, 2, 4, 8, 16] if (b == 0 and j == 0) else [0, 4, 8, 12, 16]
                    for lo, hi in zip(bounds[:-1], bounds[1:]):
                        nc.sync.dma_start(
                            out=xt[:, lo:hi, :], in_=xsrc[:, lo:hi, :]
                        )
                    xts.append(xt)

                for j in range(NJ):
                    xt = xts[j]

                    # Q^T projection, two heads per matmul (M=128)
                    qts = []
                    for p in range(PAIRS):
                        pq = prjp.tile([128, QB], F32, tag="proj")
                        for dk in range(ND):
                            nc.tensor.matmul(
                                pq[:],
                                wq_sb[:, dk, p * 128 : (p + 1) * 128],
                                xt[:, dk, :],
                                start=(dk == 0),
                                stop=(dk == ND - 1),
                            )
                        qt = qtp.tile([128, QB], F16, tag="qt")
                        nc.vector.tensor_copy(qt[:], pq[:])
                        qts.append(qt)

                    # K^T rows 0:64, V^T rows 64:128 in one accumulation
                    pkv = prjp.tile([128, QB], F32, tag="proj")
                    for dk in range(ND):
                        nc.tensor.matmul(
                            pkv[:],
                            wkv_sb[:, dk, :],
                            xt[:, dk, :],
                            start=(dk == 0),
                            stop=(dk == ND - 1),
                        )
                    nc.vector.tensor_copy(kt[0:64, j * QB : (j + 1) * QB], pkv[0:64, :])
                    nc.vector.tensor_copy(
                        kt[64:128, j * QB : (j + 1) * QB], pkv[0:64, :]
                    )
                    vt_sb = smallp.tile([64, QB], F32, tag="vt")
                    nc.scalar.copy(vt_sb[:], pkv[64:128, :])
                    for i in range(4):
                        pvt = prjp.tile([128, 64], F32, tag="proj")
                        nc.tensor.transpose(
                            pvt[:], vt_sb[:, i * 128 : (i + 1) * 128], ident[0:64, 0:64]
                        )
                        kk = 4 * j + i
                        nc.vector.tensor_copy(vv[:, kk * VW : kk * VW + DH], pvt[:])

                    if fp_queue and len(fp_queue) > 1:
                        emit_fp(*fp_queue.pop(0))

                    # attention for this query block
                    rr4 = rrp.tile([97, QB], F32, tag="rr4")
                    ous = []
                    for p in range(PAIRS):
                        po = pop.tile([65, 2 * QB], F32, tag="po")
                        nkk = 4 * (j + 1)

                        def emit_scores(kk, p=p):
                            m = kk - 4 * j
                            q0 = max(0, m) * KB  # masked-out query prefix
                            qn = QB - q0
                            ps = psp.tile([128, 2 * QB], F32, tag="ps", name="ps")
                            nc.tensor.matmul(
                                ps[:, q0:QB],
                                kt[0:64, kk * KB : (kk + 1) * KB],
                                qts[p][0:64, q0:QB],
                                start=True,
                                stop=True,
                            )
                            nc.tensor.matmul(
                                ps[:, QB + q0 : 2 * QB],
                                kt[64:128, kk * KB : (kk + 1) * KB],
                                qts[p][64:128, q0:QB],
                                start=True,
                                stop=True,
                                tile_position=(64, 0),
                            )
                            es = esp.tile([128, 2 * QB], F16, tag="es", name="es")
                            if q0 <= KB:
                                nc.scalar.activation(es[:], ps[:], EXP)
                            else:
                                nc.scalar.activation(es[:, q0:QB], ps[:, q0:QB], EXP)
                                nc.scalar.activation(
                                    es[:, QB + q0 : 2 * QB], ps[:, QB + q0 : 2 * QB], EXP
                                )
                            if m >= 0:
                                esv = es[:].rearrange("p (h q) -> p h q", q=QB)[
                                    :, :, q0:QB
                                ]
                                nc.vector.tensor_mul(
                                    esv,
                                    esv,
                                    mask_sb[:, m : m + 1, q0:QB].broadcast_to(
                                        [128, 2, qn]
                                    ),
                                )
                            return es, q0

                        es_next = emit_scores(0)
                        for kk in range(nkk):
                            es_cur, q0 = es_next
                            if kk + 1 < nkk:
                                es_next = emit_scores(kk + 1)
                            for hh in range(2):
                                nc.tensor.matmul(
                                    po[0:65, hh * QB + q0 : (hh + 1) * QB],
                                    vv[:, kk * VW : (kk + 1) * VW],
                                    es_cur[:, hh * QB + q0 : (hh + 1) * QB],
                                    start=(kk == 0),
                                    stop=(kk == nkk - 1),
                                )
                        # free PSUM fast: row sums to rr4, O' to SBUF
                        nc.vector.tensor_copy(rr4[64 * p : 64 * p + 1, :], po[64:65, 0:QB])
                        nc.vector.tensor_copy(
                            rr4[64 * p + 32 : 64 * p + 33, :], po[64:65, QB : 2 * QB]
                        )
                        ou = oup.tile([64, 2 * QB], F32, tag="ou")
                        nc.vector.tensor_copy(ou[:], po[0:64, :])
                        ous.append(ou)
                    # one reciprocal for all 4 (head, half) combos of block j
                    ir4f = rrp.tile([97, QB], F32, tag="ir4f")
                    nc.vector.reciprocal_approx_fast(ir4f[:], rr4[:])
                    ir4 = rrp.tile([97, QB], F16, tag="ir4")
                    nc.vector.tensor_copy(ir4[:], ir4f[:])
                    for p in range(PAIRS):
                        for hh in range(2):
                            k32 = 64 * p + 32 * hh
                            pb = pop.tile([64, QB], F32, tag="po")
                            nc.tensor.matmul(
                                pb[:],
                                onecol[k32 : k32 + 1, :],
                                ir4[k32 : k32 + 1, :],
                                start=True,
                                stop=True,
                                tile_position=(k32, 0),
                            )
                            hidx = 2 * p + hh
                            nc.vector.tensor_mul(
                                ctxl[
                                    64 * (hidx % 2) : 64 * (hidx % 2) + 64,
                                    hidx // 2,
                                    j * QB : (j + 1) * QB,
                                ],
                                ous[p][:, hh * QB : (hh + 1) * QB],
                                pb[:],
                            )

                    fp_queue.append((b, j))
            while fp_queue:
                emit_fp(*fp_queue.pop(0))
    nc.finalize()
    return nc


def _prep_in_maps(x, wq, wk, wv, wo):
    xT = np.ascontiguousarray(x.transpose(0, 2, 1)).astype(np.float16)
    k_idx = np.arange(128)[:, None]
    q_idx = np.arange(QB)[None, :]
    masks = np.stack(
        [(128 * m + k_idx <= q_idx).astype(np.float16) for m in range(4)]
    )
    in_maps = []
    for c in range(NC):
        wq_c = (wq[c * 256 : (c + 1) * 256] * np.float32(DH ** -0.5)).astype(np.float16)
        wkv_c = np.concatenate(
            [wk[c * DH : (c + 1) * DH], wv[c * DH : (c + 1) * DH]], axis=0
        ).astype(np.float16)
        wo_c = wo[:, c * 256 : (c + 1) * 256].astype(np.float16)  # [2048, 256]
        in_maps.append(
            {
                "xT": xT,
                "wqT": np.ascontiguousarray(wq_c.T),
                "wkvT": np.ascontiguousarray(wkv_c.T),
                "woT": np.ascontiguousarray(wo_c.T),  # [256, 2048]
                "masks": masks,
            }
        )
    return in_maps


def run(inputs, trace=False, trace_kwargs=None):
    if "nc" not in _CACHE:
        _CACHE["nc"] = _build()
    nc = _CACHE["nc"]
    in_maps = _prep_in_maps(
        np.asarray(inputs["x"], np.float32),
        np.asarray(inputs["wq"], np.float32),
        np.asarray(inputs["wk"], np.float32),
        np.asarray(inputs["wv"], np.float32),
        np.asarray(inputs["wo"], np.float32),
    )
    res = run_bass_kernel_spmd(
        nc,
        in_maps,
        core_ids=list(range(NC)),
        trace=trace,
        **(trace_kwargs or {}),
    )
    acc = np.zeros((B, D, T), np.float32)
    for r in res.results:
        acc += r["outT"]
    full = acc.transpose(0, 2, 1)
    return np.ascontiguousarray(full), res


def kernel(**inputs) -> np.ndarray:
    out, _ = run(inputs, trace=False)
    return out
